# revision 6
# baseline (speedup 1.0000x reference)
"""Trainium2 Bass kernel for nn_ActionEncoder (moe_routing).

Math (derived from the reference):
  For sample b with t = action_types[b], i0, i1 = action_indecies[b]:
    u = t*64 + i0,  v = t*64 + i1        (u, v in [0, 128))
    z[o]  = C[o, u] + D[o, v]
    out[b] = tanh(z)
  where C = [W0 | W1[:, :64]]  (4 x 128)
        D = [b0 replicated 64x | W1[:, 64:] + b1]  (4 x 128)
  (Type-0 samples hit the zero-padded half of W0 in the reference, which is
   equivalent to gathering only W0[:, i0]; biases are folded into D's columns.)

Device strategy (pure data parallel over 8 cores, 65536 samples each):
  - samples laid out [128 partitions, S=512] (partition p = samples p*S..p*S+S-1)
  - u_all/v_all computed with two fused DVE ops
  - per group g (one partition's S samples):
      PE broadcast of row g via identity-column selector matmul (K=128)
      DVE is_equal vs per-partition iota -> one-hot [128, S] in SBUF
      PE gather matmuls: psum[4,S] = C_T.T @ oh_u + D_T.T @ oh_v
      (4 groups share one PSUM bank via output col-tiling at partitions 0/32/64/96)
      ACT tanh over the [128, S] bank -> SBUF staging
  - output written feature-major [.., 4, .., S]; host reassembles to [B, 4]
"""

import numpy as np

N_CORES = 8
P = 128

_NC_CACHE = {}


def _build_nc(b_core):
    import concourse.mybir as mybir
    from concourse import bacc
    from concourse.tile import TileContext

    f32 = mybir.dt.float32
    i32 = mybir.dt.int32
    eq = mybir.AluOpType.is_equal

    S = b_core // P
    assert S * P == b_core and S <= 512
    SG, K_, A_ = 4, 8, 4  # supergroups x banks x groups-per-bank = 128 groups

    nc = bacc.Bacc("TRN2", target_bir_lowering=False, debug=False)
    idx = nc.dram_tensor("idx", [b_core, 2], i32, kind="ExternalInput")
    typ = nc.dram_tensor("typ", [b_core], i32, kind="ExternalInput")
    # tables padded to 32 output columns (cols 4.. are zero) so the gather
    # matmuls initialize the full 32-row band of their PSUM bank slice
    tabC = nc.dram_tensor("tabC", [P, 32], f32, kind="ExternalInput")
    tabD = nc.dram_tensor("tabD", [P, 32], f32, kind="ExternalInput")
    # out[sg, a, o, k, s] = tanh(z)_o for sample (sg*32 + k*4 + a)*S + s
    out = nc.dram_tensor("out", [SG, A_, 4, K_, S], f32, kind="ExternalOutput")

    with TileContext(nc) as tc:
        with tc.tile_pool(name="const", bufs=1) as cpool, \
             tc.tile_pool(name="oh", bufs=3) as ohpool, \
             tc.tile_pool(name="stage", bufs=2) as spool, \
             tc.tile_pool(name="psb", bufs=2, space="PSUM") as pbpool, \
             tc.tile_pool(name="pszp", bufs=2, space="PSUM") as pzpool:

            # ---- constants ----
            iota_pf = cpool.tile([P, P], i32, tag="iota_pf")
            nc.gpsimd.iota(iota_pf[:], pattern=[[1, P]], base=0,
                           channel_multiplier=-1)
            ident = cpool.tile([P, P], f32, tag="ident")
            nc.vector.tensor_single_scalar(ident[:], iota_pf[:], 0, eq)

            iota_i = cpool.tile([P, 1], i32, tag="iota_i")
            nc.gpsimd.iota(iota_i[:], pattern=[[1, 1]], base=0,
                           channel_multiplier=1)
            iotaF = cpool.tile([P, 1], f32, tag="iotaF")
            nc.vector.tensor_copy(out=iotaF[:], in_=iota_i[:])

            CT = cpool.tile([P, 32], f32, tag="CT")
            DT = cpool.tile([P, 32], f32, tag="DT")
            nc.sync.dma_start(out=CT[:], in_=tabC[:])
            nc.sync.dma_start(out=DT[:], in_=tabD[:])

            # ---- load + index prep ----
            I = cpool.tile([P, 2 * S], i32, tag="I")
            T = cpool.tile([P, S], i32, tag="T")
            nc.sync.dma_start(out=I[:], in_=idx.rearrange("(p n) c -> p (n c)", p=P))
            nc.sync.dma_start(out=T[:], in_=typ.rearrange("(p n) -> p n", p=P))

            I3 = I[:].rearrange("p (n c) -> p c n", c=2)
            u_all = cpool.tile([P, S], f32, tag="u_all")
            v_all = cpool.tile([P, S], f32, tag="v_all")
            t64 = cpool.tile([P, S], f32, tag="t64")
            i0f = cpool.tile([P, S], f32, tag="i0f")
            i1f = cpool.tile([P, S], f32, tag="i1f")
            # u = t*64 + i0 ; v = t*64 + i1 (each op waits on <=1 DMA sem)
            nc.vector.tensor_single_scalar(t64[:], T[:], 64.0,
                                           mybir.AluOpType.mult)
            nc.vector.tensor_copy(out=i0f[:], in_=I3[:, 0, :])
            nc.vector.tensor_copy(out=i1f[:], in_=I3[:, 1, :])
            nc.vector.tensor_add(out=u_all[:], in0=t64[:], in1=i0f[:])
            nc.vector.tensor_add(out=v_all[:], in0=t64[:], in1=i1f[:])

            # ---- main loop over 128 groups ----
            for sg in range(SG):
                stage = spool.tile([P, K_ * S], f32, tag="stage")
                for k in range(K_):
                    psz = pzpool.tile([P, S], f32, tag="psz")
                    for a in range(A_):
                        g = sg * 32 + k * 4 + a
                        sel = ident[:, g:g + 1].broadcast_to((P, P))
                        psu = pbpool.tile([P, S], f32, tag="psu")
                        psv = pbpool.tile([P, S], f32, tag="psv")
                        nc.tensor.matmul(psu[:], lhsT=sel, rhs=u_all[:],
                                         start=True, stop=True)
                        nc.tensor.matmul(psv[:], lhsT=sel, rhs=v_all[:],
                                         start=True, stop=True)
                        ohu = ohpool.tile([P, S], f32, tag="ohu")
                        ohv = ohpool.tile([P, S], f32, tag="ohv")
                        nc.vector.tensor_single_scalar(ohu[:], psu[:], iotaF[:], eq)
                        nc.vector.tensor_single_scalar(ohv[:], psv[:], iotaF[:], eq)
                        nc.tensor.matmul(psz[32 * a:32 * a + 32, :], lhsT=CT[:],
                                         rhs=ohu[:], start=True, stop=False,
                                         tile_position=(0, 32 * a),
                                         skip_group_check=True)
                        nc.tensor.matmul(psz[32 * a:32 * a + 32, :], lhsT=DT[:],
                                         rhs=ohv[:], start=False, stop=True,
                                         tile_position=(0, 32 * a),
                                         skip_group_check=True)
                    nc.scalar.activation(
                        out=stage[:, k * S:(k + 1) * S], in_=psz[:],
                        func=mybir.ActivationFunctionType.Tanh)
                for a in range(A_):
                    src = stage[32 * a:32 * a + 4, :].rearrange(
                        "p (k s) -> p k s", s=S)
                    nc.sync.dma_start(out=out[sg, a], in_=src)

    nc.compile()
    return nc, (SG, A_, K_, S)


def _tables(W0, b0, W1, b1):
    W0 = np.asarray(W0, np.float32)
    W1 = np.asarray(W1, np.float32)
    b0 = np.asarray(b0, np.float32).reshape(-1)
    b1 = np.asarray(b1, np.float32).reshape(-1)
    tabC = np.zeros((128, 32), np.float32)
    tabD = np.zeros((128, 32), np.float32)
    tabC[:, :4] = np.concatenate([W0.T, W1[:, :64].T], axis=0)
    tabD[:, :4] = np.concatenate([np.tile(b0, (64, 1)), W1[:, 64:].T + b1],
                                 axis=0)
    return tabC, tabD


def kernel(action_indecies, action_n_obj, action_types, W0, b0, W1, b1,
           **_unused):
    from concourse.bass_utils import run_bass_kernel_spmd

    idx = np.ascontiguousarray(np.asarray(action_indecies, dtype=np.int32))
    typ = np.ascontiguousarray(np.asarray(action_types, dtype=np.int32))
    B = idx.shape[0]
    b_core = B // N_CORES
    assert b_core * N_CORES == B

    tabC, tabD = _tables(W0, b0, W1, b1)

    key = b_core
    if key not in _NC_CACHE:
        _NC_CACHE[key] = _build_nc(b_core)
    nc, (SG, A_, K_, S) = _NC_CACHE[key]

    in_maps = [
        {"idx": idx[k * b_core:(k + 1) * b_core],
         "typ": typ[k * b_core:(k + 1) * b_core],
         "tabC": tabC, "tabD": tabD}
        for k in range(N_CORES)
    ]
    res = run_bass_kernel_spmd(nc, in_maps, core_ids=list(range(N_CORES)))

    outs = []
    for r in res.results:
        o5 = r["out"]  # [SG, A, 4, K, S]
        # sample (sg*32 + k*4 + a)*S + s ; feature o
        o = np.transpose(o5, (0, 3, 1, 4, 2)).reshape(b_core, 4)
        outs.append(o)
    return np.ascontiguousarray(np.concatenate(outs, axis=0))


# revision 16
# speedup vs baseline: 2.3840x; 2.3840x over previous
"""Trainium2 Bass kernel for nn_ActionEncoder (moe_routing).

Math (derived from the reference):
  For sample b with t = action_types[b], i0, i1 = action_indecies[b]:
    type 0: out = tanh(W0[:, i0] + b0)
    type 1: out = tanh(W1[:, i0] + W1[:, 64 + i1] + b1)
  This equals  out = tanh(T0 @ oh0 + T1 @ oh1)  with the reference's 128-wide
  one-hot marks at {i0, 64+i1}, type-masked:
    T0 = [W0 + b0/2 | b0/2 replicated]          (4 x 128)
    T1 = [W1[:, :64] + b1/2 | W1[:, 64:] + b1/2] (4 x 128)
    oh0 = marks if t == 0 else 0 ; oh1 = marks if t == 1 else 0

Device pipeline (pure data parallel, 8 cores x 65536 samples):
  - DMA loads indices interleaved: partition 2j   <- i0 of group j
                                   partition 2j+1 <- i1 of group j
    (group = 512 consecutive samples); types replicated the same way.
  - one fused DVE op per half:  IP = idx + 128*t   (fp16, exact: values < 256)
  - per group (512 samples):
      1 PE matmul "packed broadcast": selector picks partition pair (2j, 2j+1)
        -> psum[128, 512]: rows 0-63 = i0+128t, rows 64-127 = i1+128t
      1 copy psum -> SBUF fp16 (alternating ACT/DVE to balance engines)
      2 DVE is_equal (4x mode, 16-bit):
        oh0 = (raw == iota2),  oh1 = (raw == iota2 + 128)
        where iota2[d] = d & 63  (so rows 0-63 match i0, rows 64-127 match i1;
        the +128t shift makes each compare type-exclusive)
      2 PE table matmuls accumulate z into a shared psum bank
        (4 groups per bank via output col-tiling at partitions 0/32/64/96;
         tables padded to 32 rows so the whole band is written)
      ACT tanh over the full bank -> staging
  - output written feature-major; host reassembles to [B, 4]

TABLE_MODE: "f16"       -> single-pass fp16 tables (~5e-4 rel err, fastest)
            "bf16_hilo" -> bf16 hi + bf16 lo accumulation (~1e-6, 2 extra mms)
"""

import os

import numpy as np

N_CORES = 8
P = 128
TABLE_MODE = os.environ.get("ACTENC_TABLE_MODE", "f16")

_NC_CACHE = {}


def _build_nc(b_core, table_mode):
    import concourse.mybir as mybir
    from concourse import bacc
    from concourse.tile import TileContext

    f32 = mybir.dt.float32
    i32 = mybir.dt.int32
    i16 = mybir.dt.int16
    f16 = mybir.dt.float16 if table_mode == "f16" else mybir.dt.bfloat16
    hilo = table_mode == "bf16_hilo"
    eq = mybir.AluOpType.is_equal

    S = b_core // P
    assert S * P == b_core and S <= 512
    G = 128                # groups of S samples; group j holds samples j*S..
    GH = 64                # groups per "half" (two partitions per group)
    SG, K_, A_ = 4, 8, 4   # psz supergroups x banks x groups-per-bank

    nc = bacc.Bacc("TRN2", target_bir_lowering=False, debug=False)
    idx = nc.dram_tensor("idx", [b_core, 2], i32, kind="ExternalInput")
    typ = nc.dram_tensor("typ", [b_core], i32, kind="ExternalInput")
    ntab = 64 if hilo else 32
    tab0 = nc.dram_tensor("tab0", [P, ntab], f16, kind="ExternalInput")
    tab1 = nc.dram_tensor("tab1", [P, ntab], f16, kind="ExternalInput")
    # out[sg, a, o, k, s] = tanh(z)_o of sample (sg*32 + k*4 + a)*S + s
    out = nc.dram_tensor("out", [SG, A_, 4, K_, S], f32, kind="ExternalOutput")

    idx3 = idx.rearrange("(h g s) c -> h c g s", h=2, s=S)   # [2, 2, GH, S]
    typ3 = typ.rearrange("(h g s) -> h g s", h=2, s=S)       # [2, GH, S]

    with TileContext(nc) as tc:
        with tc.tile_pool(name="const", bufs=1) as cpool, \
             tc.tile_pool(name="oh", bufs=3) as ohpool, \
             tc.tile_pool(name="raws", bufs=3) as rpool, \
             tc.tile_pool(name="stage", bufs=2) as spool, \
             tc.tile_pool(name="psb", bufs=3, space="PSUM") as pbpool, \
             tc.tile_pool(name="pszp", bufs=2, space="PSUM") as pzpool:

            # ---- constants ----
            # selector band: view Q[:, 128j : 128j+128] = [e_j x64 | e_{64+j} x64]
            #   top half    (k <  64): 1 iff f in [128k,        128k+64)
            #   bottom half (k >= 64): 1 iff f in [128(k-64)+64, 128(k-64)+128)
            NQ = 128 * 64
            iq = cpool.tile([P, NQ], i16, tag="iq")
            nc.gpsimd.iota(iq[0:64, :], pattern=[[1, NQ]], base=0,
                           channel_multiplier=-128)
            nc.gpsimd.iota(iq[64:128, :], pattern=[[1, NQ]], base=-64,
                           channel_multiplier=-128)
            iq2 = cpool.tile([P, NQ], i16, tag="iq2")
            nc.vector.tensor_single_scalar(iq2[:], iq[:], ~63,
                                           mybir.AluOpType.bitwise_and)
            Q = cpool.tile([P, NQ], f16, tag="Q")
            nc.vector.tensor_single_scalar(Q[:], iq2[:], 0, eq)

            # iota2[d] = d mod 64 ; iota2hi = iota2 + 128
            ic = cpool.tile([P, 1], i32, tag="ic")
            nc.gpsimd.iota(ic[0:64, :], pattern=[[1, 1]], base=0,
                           channel_multiplier=1)
            nc.gpsimd.iota(ic[64:128, :], pattern=[[1, 1]], base=0,
                           channel_multiplier=1)
            iota2 = cpool.tile([P, 1], f32, tag="iota2")
            iota2hi = cpool.tile([P, 1], f32, tag="iota2hi")
            nc.vector.tensor_single_scalar(iota2[:], ic[:], 0.0,
                                           mybir.AluOpType.add)
            nc.vector.tensor_single_scalar(iota2hi[:], ic[:], 128.0,
                                           mybir.AluOpType.add)

            T0 = cpool.tile([P, ntab], f16, tag="T0")
            T1 = cpool.tile([P, ntab], f16, tag="T1")
            nc.sync.dma_start(out=T0[:], in_=tab0[:])
            nc.sync.dma_start(out=T1[:], in_=tab1[:])

            # ---- load indices/types: i0 -> partitions 0-63 (row = group),
            #      i1 -> partitions 64-127 ----
            IPraw = [cpool.tile([P, S], i32, tag=f"IPraw{h}", name=f"IPraw{h}") for h in range(2)]
            IPT = [cpool.tile([P, S], i32, tag=f"IPT{h}", name=f"IPT{h}") for h in range(2)]
            for h in range(2):
                for c in range(2):
                    nc.sync.dma_start(out=IPraw[h][64 * c:64 * c + 64, :],
                                      in_=idx3[h, c])
                    nc.sync.dma_start(out=IPT[h][64 * c:64 * c + 64, :],
                                      in_=typ3[h])

            # IP = idx + 128*t  (fp16/bf16; values <= 255 exact)
            IP = [cpool.tile([P, S], f16, tag=f"IP{h}", name=f"IP{h}") for h in range(2)]
            for h in range(2):
                nc.vector.scalar_tensor_tensor(
                    out=IP[h][:], in0=IPT[h][:], scalar=128.0,
                    in1=IPraw[h][:], op0=mybir.AluOpType.mult,
                    op1=mybir.AluOpType.add)

            # ---- main loop over 128 groups ----
            for sg in range(SG):
                stage = spool.tile([P, K_ * S], f32, tag="stage")
                for k in range(K_):
                    psz = pzpool.tile([P, S], f32, tag="psz")
                    for a in range(A_):
                        g = sg * 32 + k * 4 + a
                        h, j = divmod(g, GH)
                        sel = Q[:, 128 * j:128 * j + 128]
                        psb = pbpool.tile([P, S], f32, tag="psb")
                        nc.tensor.matmul(psb[:], lhsT=sel, rhs=IP[h][:],
                                         start=True, stop=True)
                        raw = rpool.tile([P, S], f16, tag="raw")
                        if g % 3 == 2:
                            nc.vector.tensor_copy(out=raw[:], in_=psb[:])
                        else:
                            nc.scalar.copy(raw[:], psb[:])
                        oh0 = ohpool.tile([P, S], f16, tag="oh0")
                        oh1 = ohpool.tile([P, S], f16, tag="oh1")
                        nc.vector.tensor_single_scalar(oh0[:], raw[:],
                                                       iota2[:], eq)
                        nc.vector.tensor_single_scalar(oh1[:], raw[:],
                                                       iota2hi[:], eq)
                        pz = psz[32 * a:32 * a + 32, :]
                        tp = (0, 32 * a)
                        if hilo:
                            nc.tensor.matmul(pz, lhsT=T0[:, 0:32], rhs=oh0[:],
                                             start=True, stop=False,
                                             tile_position=tp,
                                             skip_group_check=True)
                            nc.tensor.matmul(pz, lhsT=T0[:, 32:64], rhs=oh0[:],
                                             start=False, stop=False,
                                             tile_position=tp,
                                             skip_group_check=True)
                            nc.tensor.matmul(pz, lhsT=T1[:, 0:32], rhs=oh1[:],
                                             start=False, stop=False,
                                             tile_position=tp,
                                             skip_group_check=True)
                            nc.tensor.matmul(pz, lhsT=T1[:, 32:64], rhs=oh1[:],
                                             start=False, stop=True,
                                             tile_position=tp,
                                             skip_group_check=True)
                        else:
                            nc.tensor.matmul(pz, lhsT=T0[:], rhs=oh0[:],
                                             start=True, stop=False,
                                             tile_position=tp,
                                             skip_group_check=True)
                            nc.tensor.matmul(pz, lhsT=T1[:], rhs=oh1[:],
                                             start=False, stop=True,
                                             tile_position=tp,
                                             skip_group_check=True)
                    nc.scalar.activation(
                        out=stage[:, k * S:(k + 1) * S], in_=psz[:],
                        func=mybir.ActivationFunctionType.Tanh)
                for a in range(A_):
                    src = stage[32 * a:32 * a + 4, :].rearrange(
                        "p (k s) -> p k s", s=S)
                    nc.sync.dma_start(out=out[sg, a], in_=src)

    nc.compile()
    return nc, (SG, A_, K_, S)


def _tables(W0, b0, W1, b1, table_mode):
    import ml_dtypes
    W0 = np.asarray(W0, np.float32)
    W1 = np.asarray(W1, np.float32)
    b0 = np.asarray(b0, np.float32).reshape(-1)
    b1 = np.asarray(b1, np.float32).reshape(-1)
    # full-precision tables [128, 4] (row = one-hot position, col = feature)
    T0 = np.concatenate([W0.T + b0 / 2, np.tile(b0 / 2, (64, 1))], axis=0)
    T1 = np.concatenate([W1[:, :64].T + b1 / 2, W1[:, 64:].T + b1 / 2], axis=0)
    if table_mode == "f16":
        dt = ml_dtypes.float16 if hasattr(ml_dtypes, "float16") else np.float16
        out0 = np.zeros((128, 32), np.float16)
        out1 = np.zeros((128, 32), np.float16)
        out0[:, :4] = T0.astype(np.float16)
        out1[:, :4] = T1.astype(np.float16)
        return out0, out1
    else:
        bf = ml_dtypes.bfloat16
        out0 = np.zeros((128, 64), bf)
        out1 = np.zeros((128, 64), bf)
        hi0 = T0.astype(bf)
        hi1 = T1.astype(bf)
        out0[:, 0:4] = hi0
        out0[:, 32:36] = (T0 - hi0.astype(np.float32)).astype(bf)
        out1[:, 0:4] = hi1
        out1[:, 32:36] = (T1 - hi1.astype(np.float32)).astype(bf)
        return out0, out1


def kernel(action_indecies, action_n_obj, action_types, W0, b0, W1, b1,
           **_unused):
    from concourse.bass_utils import run_bass_kernel_spmd

    idx = np.ascontiguousarray(np.asarray(action_indecies, dtype=np.int32))
    typ = np.ascontiguousarray(np.asarray(action_types, dtype=np.int32))
    B = idx.shape[0]
    b_core = B // N_CORES
    assert b_core * N_CORES == B

    tab0, tab1 = _tables(W0, b0, W1, b1, TABLE_MODE)

    key = (b_core, TABLE_MODE)
    if key not in _NC_CACHE:
        _NC_CACHE[key] = _build_nc(b_core, TABLE_MODE)
    nc, (SG, A_, K_, S) = _NC_CACHE[key]

    in_maps = [
        {"idx": idx[k * b_core:(k + 1) * b_core],
         "typ": typ[k * b_core:(k + 1) * b_core],
         "tab0": tab0, "tab1": tab1}
        for k in range(N_CORES)
    ]
    res = run_bass_kernel_spmd(nc, in_maps, core_ids=list(range(N_CORES)))

    outs = []
    for r in res.results:
        o5 = r["out"]  # [SG, A, 4, K, S]
        o = np.transpose(o5, (0, 3, 1, 4, 2)).reshape(b_core, 4)
        outs.append(o)
    return np.ascontiguousarray(np.concatenate(outs, axis=0))


# revision 34
# speedup vs baseline: 3.8672x; 1.6221x over previous
"""Trainium2 Bass kernel for nn_ActionEncoder (moe_routing).

Math (derived from the reference):
  For sample b with t = action_types[b], i0, i1 = action_indecies[b]:
    type 0: out = tanh(W0[:, i0] + b0)
    type 1: out = tanh(W1[:, i0] + W1[:, 64 + i1] + b1)
  This equals  out = tanh(T0 @ oh0 + T1 @ oh1)  with the reference's 128-wide
  one-hot marks at {i0, 64+i1}, type-masked:
    T0 = [W0 + b0/2 | b0/2 replicated]          (4 x 128)
    T1 = [W1[:, :64] + b1/2 | W1[:, 64:] + b1/2] (4 x 128)
    oh0 = marks if t == 0 else 0 ; oh1 = marks if t == 1 else 0

Device pipeline (pure data parallel, 8 cores x 65536 samples):
  - DMA loads indices interleaved: partition 2j   <- i0 of group j
                                   partition 2j+1 <- i1 of group j
    (group = 512 consecutive samples); types replicated the same way.
  - one fused DVE op per half:  IP = idx + 128*t   (fp16, exact: values < 256)
  - per group (512 samples):
      1 PE matmul "packed broadcast": selector picks partition pair (2j, 2j+1)
        -> psum[128, 512]: rows 0-63 = i0+128t, rows 64-127 = i1+128t
      1 copy psum -> SBUF fp16 (alternating ACT/DVE to balance engines)
      2 DVE is_equal (4x mode, 16-bit):
        oh0 = (raw == iota2),  oh1 = (raw == iota2 + 128)
        where iota2[d] = d & 63  (so rows 0-63 match i0, rows 64-127 match i1;
        the +128t shift makes each compare type-exclusive)
      2 PE table matmuls accumulate z into a shared psum bank
        (4 groups per bank via output col-tiling at partitions 0/32/64/96;
         tables padded to 32 rows so the whole band is written)
      ACT tanh over the full bank -> staging
  - output written feature-major; host reassembles to [B, 4]

TABLE_MODE: "f16"       -> single-pass fp16 tables (~5e-4 rel err, fastest)
            "bf16_hilo" -> bf16 hi + bf16 lo accumulation (~1e-6, 2 extra mms)
"""

import os

import numpy as np

N_CORES = 8
P = 128
TABLE_MODE = os.environ.get("ACTENC_TABLE_MODE", "f16")

_NC_CACHE = {}


def _build_nc(b_core, table_mode):
    import concourse.mybir as mybir
    from concourse import bacc
    from concourse.tile import TileContext

    f32 = mybir.dt.float32
    i32 = mybir.dt.int32
    i16 = mybir.dt.int16
    f16 = mybir.dt.float16 if table_mode == "f16" else mybir.dt.bfloat16
    hilo = table_mode == "bf16_hilo"
    eq = mybir.AluOpType.is_equal

    S = b_core // P
    assert S * P == b_core and S <= 512
    G = 128                # groups of S samples; group j holds samples j*S..
    GH = 64                # groups per "half" (two partitions per group)
    SG, K_, A_ = 4, 8, 4   # psz supergroups x banks x groups-per-bank

    nc = bacc.Bacc("TRN2", target_bir_lowering=False, debug=False)
    idx = nc.dram_tensor("idx", [b_core, 2], i32, kind="ExternalInput")
    typ = nc.dram_tensor("typ", [b_core], i32, kind="ExternalInput")
    ntab = 64 if hilo else 32
    tab0 = nc.dram_tensor("tab0", [P, ntab], f16, kind="ExternalInput")
    tab1 = nc.dram_tensor("tab1", [P, ntab], f16, kind="ExternalInput")
    selq = nc.dram_tensor("selq", [P, 128 * 64], f16, kind="ExternalInput")
    # out[sg, a, o, k, s] = tanh(z)_o of sample (sg*32 + k*4 + a)*S + s
    out = nc.dram_tensor("out", [SG, A_, 4, K_, S], f32, kind="ExternalOutput")

    # pair-contiguous DRAM view: [2 halves, GH groups, 2*S] (4KB rows)
    idxp = idx.rearrange("(h g s) c -> h g (s c)", h=2, s=S)
    typ3 = typ.rearrange("(h g s) -> h g s", h=2, s=S)       # [2, GH, S]

    with TileContext(nc) as tc:
        with tc.tile_pool(name="const", bufs=1) as cpool, \
             tc.tile_pool(name="oh", bufs=3) as ohpool, \
             tc.tile_pool(name="raws", bufs=3) as rpool, \
             tc.tile_pool(name="stage", bufs=2) as spool, \
             tc.tile_pool(name="psb", bufs=3, space="PSUM") as pbpool, \
             tc.tile_pool(name="pszp", bufs=2, space="PSUM") as pzpool:

            # ---- constants ----
            # selector band (host-shipped): view Q[:, 128j : 128j+128]
            # = [e_j x64 | e_{64+j} x64]
            NQ = 128 * 64
            Q = cpool.tile([P, NQ], f16, tag="Q")
            nc.sync.dma_start(out=Q[:, 0:NQ // 2], in_=selq[:, 0:NQ // 2])
            nc.sync.dma_start(out=Q[:, NQ // 2:], in_=selq[:, NQ // 2:])

            # iota2[d] = d mod 64 ; iota2hi = iota2 + 128
            ic = cpool.tile([P, 1], i32, tag="ic")
            nc.gpsimd.iota(ic[0:64, :], pattern=[[1, 1]], base=0,
                           channel_multiplier=1)
            nc.gpsimd.iota(ic[64:128, :], pattern=[[1, 1]], base=0,
                           channel_multiplier=1)
            iota2 = cpool.tile([P, 1], f32, tag="iota2")
            iota2hi = cpool.tile([P, 1], f32, tag="iota2hi")
            nc.vector.tensor_single_scalar(iota2[:], ic[:], 0.0,
                                           mybir.AluOpType.add)
            nc.vector.tensor_single_scalar(iota2hi[:], ic[:], 128.0,
                                           mybir.AluOpType.add)

            T0 = cpool.tile([P, ntab], f16, tag="T0")
            T1 = cpool.tile([P, ntab], f16, tag="T1")
            nc.sync.dma_start(out=T0[:], in_=tab0[:])
            nc.sync.dma_start(out=T1[:], in_=tab1[:])

            # ---- load index pairs (contiguous 4KB rows) duplicated to both
            #      partition halves; types likewise.  Chunked by 32 groups so
            #      early supergroups start before all input has landed. ----
            IPraw = [cpool.tile([P, 2 * S], i32, tag=f"IPraw{h}", name=f"IPraw{h}") for h in range(2)]
            IPT = [cpool.tile([P, S], i32, tag=f"IPT{h}", name=f"IPT{h}") for h in range(2)]
            IP = [cpool.tile([P, S], f16, tag=f"IP{h}", name=f"IP{h}") for h in range(2)]
            for h in range(2):
                prw3 = IPraw[h][:].rearrange("p (s c) -> p c s", c=2)
                for jc in range(2):
                    gsl = slice(32 * jc, 32 * jc + 32)
                    for c in range(2):
                        rsl = slice(64 * c + 32 * jc, 64 * c + 32 * jc + 32)
                        nc.sync.dma_start(out=IPraw[h][rsl, :],
                                          in_=idxp[h, gsl])
                        nc.sync.dma_start(out=IPT[h][rsl, :],
                                          in_=typ3[h, gsl])
                        nc.vector.scalar_tensor_tensor(
                            out=IP[h][rsl, :],
                            in0=IPT[h][rsl, :], scalar=128.0,
                            in1=prw3[rsl, c, :],
                            op0=mybir.AluOpType.mult,
                            op1=mybir.AluOpType.add)

            # ---- main loop: quads of 4 groups; psb double-buffered pairs ----
            for sg in range(SG):
                stage = spool.tile([P, K_ * S], f32, tag="stage")
                for k in range(K_):
                    psz = pzpool.tile([P, S], f32, tag="psz")
                    raw = rpool.tile([P, 4 * S], f16, tag="raw", name="raw")
                    for half in range(2):
                        pi = (sg * K_ + k) * 2 + half
                        # 2 broadcast matmuls into a 2-bank psum pair
                        psb = pbpool.tile([P, 2 * S], f32, tag="psb",
                                          name="psb")
                        for i in range(2):
                            a = half * 2 + i
                            g = sg * 32 + k * 4 + a
                            h, j = divmod(g, GH)
                            sel = Q[:, 128 * j:128 * j + 128]
                            nc.tensor.matmul(psb[:, i * S:(i + 1) * S],
                                             lhsT=sel, rhs=IP[h][:],
                                             start=True, stop=True)
                        # batched copy psum->sbuf fp16 (alternate ACT/DVE)
                        rsl = slice(half * 2 * S, half * 2 * S + 2 * S)
                        if pi % 4 == 3:
                            nc.vector.tensor_copy(out=raw[:, rsl], in_=psb[:])
                        else:
                            nc.scalar.copy(raw[:, rsl], psb[:])
                    # two quad-wide compares -> both type-masked one-hots
                    oh0 = ohpool.tile([P, 4 * S], f16, tag="oh0", name="oh0")
                    oh1 = ohpool.tile([P, 4 * S], f16, tag="oh1", name="oh1")
                    nc.vector.tensor_single_scalar(oh0[:], raw[:],
                                                   iota2[:], eq)
                    nc.vector.tensor_single_scalar(oh1[:], raw[:],
                                                   iota2hi[:], eq)
                    # 8 gather matmuls back-to-back accumulate z
                    for i2 in range(2):
                        for a in range(A_):
                            ohx = oh0 if i2 == 0 else oh1
                            tab = T0 if i2 == 0 else T1
                            pz = psz[32 * a:32 * a + 32, :]
                            tp = (0, 32 * a)
                            sl = slice(a * S, a * S + S)
                            if hilo:
                                nc.tensor.matmul(pz, lhsT=tab[:, 0:32],
                                                 rhs=ohx[:, sl],
                                                 start=(i2 == 0), stop=False,
                                                 tile_position=tp,
                                                 skip_group_check=True)
                                nc.tensor.matmul(pz, lhsT=tab[:, 32:64],
                                                 rhs=ohx[:, sl],
                                                 start=False, stop=(i2 == 1),
                                                 tile_position=tp,
                                                 skip_group_check=True)
                            else:
                                nc.tensor.matmul(pz, lhsT=tab[:],
                                                 rhs=ohx[:, sl],
                                                 start=(i2 == 0),
                                                 stop=(i2 == 1),
                                                 tile_position=tp,
                                                 skip_group_check=True)
                    nc.scalar.activation(
                        out=stage[:, k * S:(k + 1) * S], in_=psz[:],
                        func=mybir.ActivationFunctionType.Tanh)
                for a in range(A_):
                    src = stage[32 * a:32 * a + 4, :].rearrange(
                        "p (k s) -> p k s", s=S)
                    nc.sync.dma_start(out=out[sg, a], in_=src)

    nc.compile()
    return nc, (SG, A_, K_, S)


def _selq(table_mode):
    import ml_dtypes
    dt = np.float16 if table_mode == "f16" else ml_dtypes.bfloat16
    Q = np.zeros((128, 128 * 64), dt)
    k = np.arange(64)
    f = np.arange(128 * 64)
    top = ((f[None, :] - 128 * k[:, None]) >= 0) & \
          ((f[None, :] - 128 * k[:, None]) < 64)
    bot = ((f[None, :] - 128 * k[:, None] - 64) >= 0) & \
          ((f[None, :] - 128 * k[:, None] - 64) < 128 - 64)
    Q[0:64, :] = top.astype(dt)
    Q[64:128, :] = bot.astype(dt)
    return Q


def _tables(W0, b0, W1, b1, table_mode):
    import ml_dtypes
    W0 = np.asarray(W0, np.float32)
    W1 = np.asarray(W1, np.float32)
    b0 = np.asarray(b0, np.float32).reshape(-1)
    b1 = np.asarray(b1, np.float32).reshape(-1)
    # full-precision tables [128, 4] (row = one-hot position, col = feature)
    T0 = np.concatenate([W0.T + b0 / 2, np.tile(b0 / 2, (64, 1))], axis=0)
    T1 = np.concatenate([W1[:, :64].T + b1 / 2, W1[:, 64:].T + b1 / 2], axis=0)
    if table_mode == "f16":
        dt = ml_dtypes.float16 if hasattr(ml_dtypes, "float16") else np.float16
        out0 = np.zeros((128, 32), np.float16)
        out1 = np.zeros((128, 32), np.float16)
        out0[:, :4] = T0.astype(np.float16)
        out1[:, :4] = T1.astype(np.float16)
        return out0, out1
    else:
        bf = ml_dtypes.bfloat16
        out0 = np.zeros((128, 64), bf)
        out1 = np.zeros((128, 64), bf)
        hi0 = T0.astype(bf)
        hi1 = T1.astype(bf)
        out0[:, 0:4] = hi0
        out0[:, 32:36] = (T0 - hi0.astype(np.float32)).astype(bf)
        out1[:, 0:4] = hi1
        out1[:, 32:36] = (T1 - hi1.astype(np.float32)).astype(bf)
        return out0, out1


def kernel(action_indecies, action_n_obj, action_types, W0, b0, W1, b1,
           **_unused):
    from concourse.bass_utils import run_bass_kernel_spmd

    idx = np.ascontiguousarray(np.asarray(action_indecies, dtype=np.int32))
    typ = np.ascontiguousarray(np.asarray(action_types, dtype=np.int32))
    B = idx.shape[0]
    b_core = B // N_CORES
    assert b_core * N_CORES == B

    tab0, tab1 = _tables(W0, b0, W1, b1, TABLE_MODE)
    selq = _selq(TABLE_MODE)

    key = (b_core, TABLE_MODE)
    if key not in _NC_CACHE:
        _NC_CACHE[key] = _build_nc(b_core, TABLE_MODE)
    nc, (SG, A_, K_, S) = _NC_CACHE[key]

    in_maps = [
        {"idx": idx[k * b_core:(k + 1) * b_core],
         "typ": typ[k * b_core:(k + 1) * b_core],
         "tab0": tab0, "tab1": tab1, "selq": selq}
        for k in range(N_CORES)
    ]
    res = run_bass_kernel_spmd(nc, in_maps, core_ids=list(range(N_CORES)))

    outs = []
    for r in res.results:
        o5 = r["out"]  # [SG, A, 4, K, S]
        o = np.transpose(o5, (0, 3, 1, 4, 2)).reshape(b_core, 4)
        outs.append(o)
    return np.ascontiguousarray(np.concatenate(outs, axis=0))


# revision 37
# speedup vs baseline: 4.3084x; 1.1141x over previous
"""Trainium2 Bass kernel for nn_ActionEncoder (moe_routing).

Math (derived from the reference):
  For sample b with t = action_types[b], i0, i1 = action_indecies[b]:
    type 0: out = tanh(W0[:, i0] + b0)
    type 1: out = tanh(W1[:, i0] + W1[:, 64 + i1] + b1)
  This equals  out = tanh(T0 @ oh0 + T1 @ oh1)  with the reference's 128-wide
  one-hot marks at {i0, 64+i1}, type-masked:
    T0 = [W0 + b0/2 | b0/2 replicated]          (4 x 128)
    T1 = [W1[:, :64] + b1/2 | W1[:, 64:] + b1/2] (4 x 128)
    oh0 = marks if t == 0 else 0 ; oh1 = marks if t == 1 else 0

Device pipeline (pure data parallel, 8 cores x 65536 samples):
  - DMA loads indices interleaved: partition 2j   <- i0 of group j
                                   partition 2j+1 <- i1 of group j
    (group = 512 consecutive samples); types replicated the same way.
  - one fused DVE op per half:  IP = idx + 128*t   (fp16, exact: values < 256)
  - per group (512 samples):
      1 PE matmul "packed broadcast": selector picks partition pair (2j, 2j+1)
        -> psum[128, 512]: rows 0-63 = i0+128t, rows 64-127 = i1+128t
      1 copy psum -> SBUF fp16 (alternating ACT/DVE to balance engines)
      2 DVE is_equal (4x mode, 16-bit):
        oh0 = (raw == iota2),  oh1 = (raw == iota2 + 128)
        where iota2[d] = d & 63  (so rows 0-63 match i0, rows 64-127 match i1;
        the +128t shift makes each compare type-exclusive)
      2 PE table matmuls accumulate z into a shared psum bank
        (4 groups per bank via output col-tiling at partitions 0/32/64/96;
         tables padded to 32 rows so the whole band is written)
      ACT tanh over the full bank -> staging
  - output written feature-major; host reassembles to [B, 4]

TABLE_MODE: "f16"       -> single-pass fp16 tables (~5e-4 rel err, fastest)
            "bf16_hilo" -> bf16 hi + bf16 lo accumulation (~1e-6, 2 extra mms)
"""

import os

import numpy as np

N_CORES = 8
P = 128
TABLE_MODE = os.environ.get("ACTENC_TABLE_MODE", "bf16_hilo")

_NC_CACHE = {}


def _build_nc(b_core, table_mode):
    import concourse.mybir as mybir
    from concourse import bacc
    from concourse.tile import TileContext

    f32 = mybir.dt.float32
    i32 = mybir.dt.int32
    i16 = mybir.dt.int16
    f16 = mybir.dt.float16 if table_mode == "f16" else mybir.dt.bfloat16
    hilo = table_mode == "bf16_hilo"
    eq = mybir.AluOpType.is_equal

    S = b_core // P
    assert S * P == b_core and S <= 512
    G = 128                # groups of S samples; group j holds samples j*S..
    GH = 64                # groups per "half" (two partitions per group)
    SG, K_, A_ = 4, 8, 4   # psz supergroups x banks x groups-per-bank

    nc = bacc.Bacc("TRN2", target_bir_lowering=False, debug=False)
    idx = nc.dram_tensor("idx", [b_core, 2], i32, kind="ExternalInput")
    typ = nc.dram_tensor("typ", [b_core], i32, kind="ExternalInput")
    ntab = 64 if hilo else 32
    tab0 = nc.dram_tensor("tab0", [P, ntab], f16, kind="ExternalInput")
    tab1 = nc.dram_tensor("tab1", [P, ntab], f16, kind="ExternalInput")
    selq = nc.dram_tensor("selq", [P, 128 * 64], f16, kind="ExternalInput")
    # out[sg, a, o, k, s] = tanh(z)_o of sample (sg*32 + k*4 + a)*S + s
    out = nc.dram_tensor("out", [SG, A_, 4, K_, S], f32, kind="ExternalOutput")

    # pair-contiguous DRAM view: [2 halves, GH groups, 2*S] (4KB rows)
    idxp = idx.rearrange("(h g s) c -> h g (s c)", h=2, s=S)
    typ3 = typ.rearrange("(h g s) -> h g s", h=2, s=S)       # [2, GH, S]

    with TileContext(nc) as tc:
        with tc.tile_pool(name="const", bufs=1) as cpool, \
             tc.tile_pool(name="oh", bufs=3) as ohpool, \
             tc.tile_pool(name="raws", bufs=3) as rpool, \
             tc.tile_pool(name="stage", bufs=2) as spool, \
             tc.tile_pool(name="psb", bufs=3, space="PSUM") as pbpool, \
             tc.tile_pool(name="pszp", bufs=2, space="PSUM") as pzpool:

            # ---- constants ----
            # selector band (host-shipped): view Q[:, 128j : 128j+128]
            # = [e_j x64 | e_{64+j} x64]
            NQ = 128 * 64
            Q = cpool.tile([P, NQ], f16, tag="Q")
            nc.sync.dma_start(out=Q[:, 0:NQ // 2], in_=selq[:, 0:NQ // 2])
            nc.sync.dma_start(out=Q[:, NQ // 2:], in_=selq[:, NQ // 2:])

            # iota2[d] = d mod 64 ; iota2hi = iota2 + 128
            ic = cpool.tile([P, 1], i32, tag="ic")
            nc.gpsimd.iota(ic[0:64, :], pattern=[[1, 1]], base=0,
                           channel_multiplier=1)
            nc.gpsimd.iota(ic[64:128, :], pattern=[[1, 1]], base=0,
                           channel_multiplier=1)
            iota2 = cpool.tile([P, 1], f32, tag="iota2")
            iota2hi = cpool.tile([P, 1], f32, tag="iota2hi")
            nc.vector.tensor_single_scalar(iota2[:], ic[:], 0.0,
                                           mybir.AluOpType.add)
            nc.vector.tensor_single_scalar(iota2hi[:], ic[:], 128.0,
                                           mybir.AluOpType.add)

            T0 = cpool.tile([P, ntab], f16, tag="T0")
            T1 = cpool.tile([P, ntab], f16, tag="T1")
            nc.sync.dma_start(out=T0[:], in_=tab0[:])
            nc.sync.dma_start(out=T1[:], in_=tab1[:])

            # ---- load index pairs (contiguous 4KB rows) duplicated to both
            #      partition halves; types likewise.  Chunked by 32 groups so
            #      early supergroups start before all input has landed. ----
            IPraw = [cpool.tile([P, 2 * S], i32, tag=f"IPraw{h}", name=f"IPraw{h}") for h in range(2)]
            IPT = [cpool.tile([P, S], i32, tag=f"IPT{h}", name=f"IPT{h}") for h in range(2)]
            IP = [cpool.tile([P, S], f16, tag=f"IP{h}", name=f"IP{h}") for h in range(2)]
            for h in range(2):
                prw3 = IPraw[h][:].rearrange("p (s c) -> p c s", c=2)
                for jc in range(2):
                    gsl = slice(32 * jc, 32 * jc + 32)
                    for c in range(2):
                        rsl = slice(64 * c + 32 * jc, 64 * c + 32 * jc + 32)
                        nc.sync.dma_start(out=IPraw[h][rsl, :],
                                          in_=idxp[h, gsl])
                        nc.sync.dma_start(out=IPT[h][rsl, :],
                                          in_=typ3[h, gsl])
                        nc.vector.scalar_tensor_tensor(
                            out=IP[h][rsl, :],
                            in0=IPT[h][rsl, :], scalar=128.0,
                            in1=prw3[rsl, c, :],
                            op0=mybir.AluOpType.mult,
                            op1=mybir.AluOpType.add)

            # ---- main loop: quads of 4 groups; psb double-buffered pairs ----
            for sg in range(SG):
                stage = spool.tile([P, K_ * S], f32, tag="stage")
                for k in range(K_):
                    psz = pzpool.tile([P, S], f32, tag="psz")
                    raw = rpool.tile([P, 4 * S], f16, tag="raw", name="raw")
                    for half in range(2):
                        pi = (sg * K_ + k) * 2 + half
                        # 2 broadcast matmuls into a 2-bank psum pair
                        psb = pbpool.tile([P, 2 * S], f32, tag="psb",
                                          name="psb")
                        for i in range(2):
                            a = half * 2 + i
                            g = sg * 32 + k * 4 + a
                            h, j = divmod(g, GH)
                            sel = Q[:, 128 * j:128 * j + 128]
                            nc.tensor.matmul(psb[:, i * S:(i + 1) * S],
                                             lhsT=sel, rhs=IP[h][:],
                                             start=True, stop=True)
                        # batched copy psum->sbuf fp16 (alternate ACT/DVE)
                        rsl = slice(half * 2 * S, half * 2 * S + 2 * S)
                        if pi % 4 == 3:
                            nc.vector.tensor_copy(out=raw[:, rsl], in_=psb[:])
                        else:
                            nc.scalar.copy(raw[:, rsl], psb[:])
                    # two quad-wide compares -> both type-masked one-hots
                    oh0 = ohpool.tile([P, 4 * S], f16, tag="oh0", name="oh0")
                    oh1 = ohpool.tile([P, 4 * S], f16, tag="oh1", name="oh1")
                    nc.vector.tensor_single_scalar(oh0[:], raw[:],
                                                   iota2[:], eq)
                    nc.vector.tensor_single_scalar(oh1[:], raw[:],
                                                   iota2hi[:], eq)
                    # 8 gather matmuls back-to-back accumulate z
                    for i2 in range(2):
                        for a in range(A_):
                            ohx = oh0 if i2 == 0 else oh1
                            tab = T0 if i2 == 0 else T1
                            pz = psz[32 * a:32 * a + 32, :]
                            tp = (0, 32 * a)
                            sl = slice(a * S, a * S + S)
                            if hilo:
                                nc.tensor.matmul(pz, lhsT=tab[:, 0:32],
                                                 rhs=ohx[:, sl],
                                                 start=(i2 == 0), stop=False,
                                                 tile_position=tp,
                                                 skip_group_check=True)
                                nc.tensor.matmul(pz, lhsT=tab[:, 32:64],
                                                 rhs=ohx[:, sl],
                                                 start=False, stop=(i2 == 1),
                                                 tile_position=tp,
                                                 skip_group_check=True)
                            else:
                                nc.tensor.matmul(pz, lhsT=tab[:],
                                                 rhs=ohx[:, sl],
                                                 start=(i2 == 0),
                                                 stop=(i2 == 1),
                                                 tile_position=tp,
                                                 skip_group_check=True)
                    nc.scalar.activation(
                        out=stage[:, k * S:(k + 1) * S], in_=psz[:],
                        func=mybir.ActivationFunctionType.Tanh)
                for a in range(A_):
                    src = stage[32 * a:32 * a + 4, :].rearrange(
                        "p (k s) -> p k s", s=S)
                    nc.sync.dma_start(out=out[sg, a], in_=src)

    nc.compile()
    return nc, (SG, A_, K_, S)


def _selq(table_mode):
    import ml_dtypes
    dt = np.float16 if table_mode == "f16" else ml_dtypes.bfloat16
    Q = np.zeros((128, 128 * 64), dt)
    k = np.arange(64)
    f = np.arange(128 * 64)
    top = ((f[None, :] - 128 * k[:, None]) >= 0) & \
          ((f[None, :] - 128 * k[:, None]) < 64)
    bot = ((f[None, :] - 128 * k[:, None] - 64) >= 0) & \
          ((f[None, :] - 128 * k[:, None] - 64) < 128 - 64)
    Q[0:64, :] = top.astype(dt)
    Q[64:128, :] = bot.astype(dt)
    return Q


def _tables(W0, b0, W1, b1, table_mode):
    import ml_dtypes
    W0 = np.asarray(W0, np.float32)
    W1 = np.asarray(W1, np.float32)
    b0 = np.asarray(b0, np.float32).reshape(-1)
    b1 = np.asarray(b1, np.float32).reshape(-1)
    # full-precision tables [128, 4] (row = one-hot position, col = feature)
    T0 = np.concatenate([W0.T + b0 / 2, np.tile(b0 / 2, (64, 1))], axis=0)
    T1 = np.concatenate([W1[:, :64].T + b1 / 2, W1[:, 64:].T + b1 / 2], axis=0)
    if table_mode == "f16":
        dt = ml_dtypes.float16 if hasattr(ml_dtypes, "float16") else np.float16
        out0 = np.zeros((128, 32), np.float16)
        out1 = np.zeros((128, 32), np.float16)
        out0[:, :4] = T0.astype(np.float16)
        out1[:, :4] = T1.astype(np.float16)
        return out0, out1
    else:
        bf = ml_dtypes.bfloat16
        out0 = np.zeros((128, 64), bf)
        out1 = np.zeros((128, 64), bf)
        hi0 = T0.astype(bf)
        hi1 = T1.astype(bf)
        out0[:, 0:4] = hi0
        out0[:, 32:36] = (T0 - hi0.astype(np.float32)).astype(bf)
        out1[:, 0:4] = hi1
        out1[:, 32:36] = (T1 - hi1.astype(np.float32)).astype(bf)
        return out0, out1


def kernel(action_indecies, action_n_obj, action_types, W0, b0, W1, b1,
           **_unused):
    from concourse.bass_utils import run_bass_kernel_spmd

    idx = np.ascontiguousarray(np.asarray(action_indecies, dtype=np.int32))
    typ = np.ascontiguousarray(np.asarray(action_types, dtype=np.int32))
    B = idx.shape[0]
    b_core = B // N_CORES
    assert b_core * N_CORES == B

    tab0, tab1 = _tables(W0, b0, W1, b1, TABLE_MODE)
    selq = _selq(TABLE_MODE)

    key = (b_core, TABLE_MODE)
    if key not in _NC_CACHE:
        _NC_CACHE[key] = _build_nc(b_core, TABLE_MODE)
    nc, (SG, A_, K_, S) = _NC_CACHE[key]

    in_maps = [
        {"idx": idx[k * b_core:(k + 1) * b_core],
         "typ": typ[k * b_core:(k + 1) * b_core],
         "tab0": tab0, "tab1": tab1, "selq": selq}
        for k in range(N_CORES)
    ]
    res = run_bass_kernel_spmd(nc, in_maps, core_ids=list(range(N_CORES)))

    outs = []
    for r in res.results:
        o5 = r["out"]  # [SG, A, 4, K, S]
        o = np.transpose(o5, (0, 3, 1, 4, 2)).reshape(b_core, 4)
        outs.append(o)
    return np.ascontiguousarray(np.concatenate(outs, axis=0))


# revision 40
# speedup vs baseline: 4.3411x; 1.0076x over previous
"""Trainium2 Bass kernel for nn_ActionEncoder (moe_routing).

Math (derived from the reference):
  For sample b with t = action_types[b], i0, i1 = action_indecies[b]:
    type 0: out = tanh(W0[:, i0] + b0)
    type 1: out = tanh(W1[:, i0] + W1[:, 64 + i1] + b1)
  This equals  out = tanh(T0 @ oh0 + T1 @ oh1)  with the reference's 128-wide
  one-hot marks at {i0, 64+i1}, type-masked:
    T0 = [W0 + b0/2 | b0/2 replicated]          (4 x 128)
    T1 = [W1[:, :64] + b1/2 | W1[:, 64:] + b1/2] (4 x 128)
    oh0 = marks if t == 0 else 0 ; oh1 = marks if t == 1 else 0

Device pipeline (pure data parallel, 8 cores x 65536 samples):
  - DMA loads indices interleaved: partition 2j   <- i0 of group j
                                   partition 2j+1 <- i1 of group j
    (group = 512 consecutive samples); types replicated the same way.
  - one fused DVE op per half:  IP = idx + 128*t   (fp16, exact: values < 256)
  - per group (512 samples):
      1 PE matmul "packed broadcast": selector picks partition pair (2j, 2j+1)
        -> psum[128, 512]: rows 0-63 = i0+128t, rows 64-127 = i1+128t
      1 copy psum -> SBUF fp16 (alternating ACT/DVE to balance engines)
      2 DVE is_equal (4x mode, 16-bit):
        oh0 = (raw == iota2),  oh1 = (raw == iota2 + 128)
        where iota2[d] = d & 63  (so rows 0-63 match i0, rows 64-127 match i1;
        the +128t shift makes each compare type-exclusive)
      2 PE table matmuls accumulate z into a shared psum bank
        (4 groups per bank via output col-tiling at partitions 0/32/64/96;
         tables padded to 32 rows so the whole band is written)
      ACT tanh over the full bank -> staging
  - output written feature-major; host reassembles to [B, 4]

TABLE_MODE: "f16"       -> single-pass fp16 tables (~5e-4 rel err, fastest)
            "bf16_hilo" -> bf16 hi + bf16 lo accumulation (~1e-6, 2 extra mms)
"""

import os

import numpy as np

N_CORES = 8
P = 128
TABLE_MODE = os.environ.get("ACTENC_TABLE_MODE", "bf16_hilo")

_NC_CACHE = {}


def _build_nc(b_core, table_mode):
    import concourse.mybir as mybir
    from concourse import bacc
    from concourse.tile import TileContext

    f32 = mybir.dt.float32
    i32 = mybir.dt.int32
    i16 = mybir.dt.int16
    f16 = mybir.dt.float16 if table_mode == "f16" else mybir.dt.bfloat16
    hilo = table_mode == "bf16_hilo"
    eq = mybir.AluOpType.is_equal

    S = b_core // P
    assert S * P == b_core and S <= 512
    G = 128                # groups of S samples; group j holds samples j*S..
    GH = 64                # groups per "half" (two partitions per group)
    SG, K_, A_ = 4, 8, 4   # psz supergroups x banks x groups-per-bank

    nc = bacc.Bacc("TRN2", target_bir_lowering=False, debug=False)
    idx = nc.dram_tensor("idx", [b_core, 2], i32, kind="ExternalInput")
    typ = nc.dram_tensor("typ", [b_core], i32, kind="ExternalInput")
    ntab = 64 if hilo else 32
    tab0 = nc.dram_tensor("tab0", [P, ntab], f16, kind="ExternalInput")
    tab1 = nc.dram_tensor("tab1", [P, ntab], f16, kind="ExternalInput")
    selq = nc.dram_tensor("selq", [P, 128 * 64], f16, kind="ExternalInput")
    # out[sg, a, o, k, s] = tanh(z)_o of sample (sg*32 + k*4 + a)*S + s
    out = nc.dram_tensor("out", [SG, A_, 4, K_, S], f32, kind="ExternalOutput")

    # pair-contiguous DRAM view: [2 halves, GH groups, 2*S] (4KB rows)
    idxp = idx.rearrange("(h g s) c -> h g (s c)", h=2, s=S)
    typ3 = typ.rearrange("(h g s) -> h g s", h=2, s=S)       # [2, GH, S]

    with TileContext(nc) as tc:
        with tc.tile_pool(name="const", bufs=1) as cpool, \
             tc.tile_pool(name="oh", bufs=3) as ohpool, \
             tc.tile_pool(name="raws", bufs=3) as rpool, \
             tc.tile_pool(name="stage", bufs=2) as spool, \
             tc.tile_pool(name="psb", bufs=3, space="PSUM") as pbpool, \
             tc.tile_pool(name="pszp", bufs=2, space="PSUM") as pzpool:

            # ---- constants ----
            # selector band (host-shipped): view Q[:, 128j : 128j+128]
            # = [e_j x64 | e_{64+j} x64]
            NQ = 128 * 64
            Q = cpool.tile([P, NQ], f16, tag="Q")
            nc.sync.dma_start(out=Q[:, 0:NQ // 2], in_=selq[:, 0:NQ // 2])
            nc.sync.dma_start(out=Q[:, NQ // 2:], in_=selq[:, NQ // 2:])

            # iota2[d] = d mod 64 ; iota2hi = iota2 + 128
            ic = cpool.tile([P, 1], i32, tag="ic")
            nc.gpsimd.iota(ic[0:64, :], pattern=[[1, 1]], base=0,
                           channel_multiplier=1)
            nc.gpsimd.iota(ic[64:128, :], pattern=[[1, 1]], base=0,
                           channel_multiplier=1)
            iota2 = cpool.tile([P, 1], f32, tag="iota2")
            iota2hi = cpool.tile([P, 1], f32, tag="iota2hi")
            nc.vector.tensor_single_scalar(iota2[:], ic[:], 0.0,
                                           mybir.AluOpType.add)
            nc.vector.tensor_single_scalar(iota2hi[:], ic[:], 128.0,
                                           mybir.AluOpType.add)

            T0 = cpool.tile([P, ntab], f16, tag="T0")
            T1 = cpool.tile([P, ntab], f16, tag="T1")
            nc.sync.dma_start(out=T0[:], in_=tab0[:])
            nc.sync.dma_start(out=T1[:], in_=tab1[:])

            # ---- load index pairs (contiguous 4KB rows) duplicated to both
            #      partition halves; types likewise.  Chunked by 32 groups so
            #      early supergroups start before all input has landed. ----
            IPraw = [cpool.tile([P, 2 * S], i32, tag=f"IPraw{h}", name=f"IPraw{h}") for h in range(2)]
            IPT = [cpool.tile([P, S], i32, tag=f"IPT{h}", name=f"IPT{h}") for h in range(2)]
            IP = [cpool.tile([P, S], f16, tag=f"IP{h}", name=f"IP{h}") for h in range(2)]
            for h in range(2):
                prw3 = IPraw[h][:].rearrange("p (s c) -> p c s", c=2)
                for jc in range(2):
                    gsl = slice(32 * jc, 32 * jc + 32)
                    for c in range(2):
                        rsl = slice(64 * c + 32 * jc, 64 * c + 32 * jc + 32)
                        nc.sync.dma_start(out=IPraw[h][rsl, :],
                                          in_=idxp[h, gsl])
                        nc.sync.dma_start(out=IPT[h][rsl, :],
                                          in_=typ3[h, gsl])
                        nc.vector.scalar_tensor_tensor(
                            out=IP[h][rsl, :],
                            in0=IPT[h][rsl, :], scalar=128.0,
                            in1=prw3[rsl, c, :],
                            op0=mybir.AluOpType.mult,
                            op1=mybir.AluOpType.add)

            # ---- main loop: quads of 4 groups; psb double-buffered pairs ----
            for sg in range(SG):
                stage = spool.tile([P, K_ * S], f32, tag="stage")
                for k in range(K_):
                    psz = pzpool.tile([P, S], f32, tag="psz")
                    raw = rpool.tile([P, 4 * S], f16, tag="raw", name="raw")
                    for half in range(2):
                        pi = (sg * K_ + k) * 2 + half
                        # 2 broadcast matmuls into a 2-bank psum pair
                        psb = pbpool.tile([P, 2 * S], f32, tag="psb",
                                          name="psb")
                        for i in range(2):
                            a = half * 2 + i
                            g = sg * 32 + k * 4 + a
                            h, j = divmod(g, GH)
                            sel = Q[:, 128 * j:128 * j + 128]
                            nc.tensor.matmul(psb[:, i * S:(i + 1) * S],
                                             lhsT=sel, rhs=IP[h][:],
                                             start=True, stop=True)
                        # batched copy psum->sbuf fp16 (alternate ACT/DVE)
                        rsl = slice(half * 2 * S, half * 2 * S + 2 * S)
                        if pi % 4 == 3:
                            nc.vector.tensor_copy(out=raw[:, rsl], in_=psb[:])
                        else:
                            nc.scalar.copy(raw[:, rsl], psb[:])
                    # two quad-wide compares -> both type-masked one-hots
                    oh0 = ohpool.tile([P, 4 * S], f16, tag="oh0", name="oh0")
                    oh1 = ohpool.tile([P, 4 * S], f16, tag="oh1", name="oh1")
                    nc.vector.tensor_single_scalar(oh0[:], raw[:],
                                                   iota2[:], eq)
                    nc.vector.tensor_single_scalar(oh1[:], raw[:],
                                                   iota2hi[:], eq)
                    # 8 gather matmuls back-to-back accumulate z
                    for i2 in range(2):
                        for a in range(A_):
                            ohx = oh0 if i2 == 0 else oh1
                            tab = T0 if i2 == 0 else T1
                            pz = psz[32 * a:32 * a + 32, :]
                            tp = (0, 32 * a)
                            sl = slice(a * S, a * S + S)
                            if hilo:
                                nc.tensor.matmul(pz, lhsT=tab[:, 0:32],
                                                 rhs=ohx[:, sl],
                                                 start=(i2 == 0), stop=False,
                                                 tile_position=tp,
                                                 skip_group_check=True)
                                nc.tensor.matmul(pz, lhsT=tab[:, 32:64],
                                                 rhs=ohx[:, sl],
                                                 start=False, stop=(i2 == 1),
                                                 tile_position=tp,
                                                 skip_group_check=True)
                            else:
                                nc.tensor.matmul(pz, lhsT=tab[:],
                                                 rhs=ohx[:, sl],
                                                 start=(i2 == 0),
                                                 stop=(i2 == 1),
                                                 tile_position=tp,
                                                 skip_group_check=True)
                    nc.scalar.activation(
                        out=stage[:, k * S:(k + 1) * S], in_=psz[:],
                        func=mybir.ActivationFunctionType.Tanh)
                for a in range(A_):
                    src = stage[32 * a:32 * a + 4, :].rearrange(
                        "p (k s) -> p k s", s=S)
                    nc.sync.dma_start(out=out[sg, a], in_=src)

    nc.compile()
    return nc, (SG, A_, K_, S)


def _selq(table_mode):
    import ml_dtypes
    dt = np.float16 if table_mode == "f16" else ml_dtypes.bfloat16
    Q = np.zeros((128, 128 * 64), dt)
    k = np.arange(64)
    f = np.arange(128 * 64)
    top = ((f[None, :] - 128 * k[:, None]) >= 0) & \
          ((f[None, :] - 128 * k[:, None]) < 64)
    bot = ((f[None, :] - 128 * k[:, None] - 64) >= 0) & \
          ((f[None, :] - 128 * k[:, None] - 64) < 128 - 64)
    Q[0:64, :] = top.astype(dt)
    Q[64:128, :] = bot.astype(dt)
    return Q


def _tables(W0, b0, W1, b1, table_mode):
    import ml_dtypes
    W0 = np.asarray(W0, np.float32)
    W1 = np.asarray(W1, np.float32)
    b0 = np.asarray(b0, np.float32).reshape(-1)
    b1 = np.asarray(b1, np.float32).reshape(-1)
    # full-precision tables [128, 4] (row = one-hot position, col = feature)
    T0 = np.concatenate([W0.T + b0 / 2, np.tile(b0 / 2, (64, 1))], axis=0)
    T1 = np.concatenate([W1[:, :64].T + b1 / 2, W1[:, 64:].T + b1 / 2], axis=0)
    if table_mode == "f16":
        dt = ml_dtypes.float16 if hasattr(ml_dtypes, "float16") else np.float16
        out0 = np.zeros((128, 32), np.float16)
        out1 = np.zeros((128, 32), np.float16)
        out0[:, :4] = T0.astype(np.float16)
        out1[:, :4] = T1.astype(np.float16)
        return out0, out1
    else:
        bf = ml_dtypes.bfloat16
        out0 = np.zeros((128, 64), bf)
        out1 = np.zeros((128, 64), bf)
        hi0 = T0.astype(bf)
        hi1 = T1.astype(bf)
        out0[:, 0:4] = hi0
        out0[:, 32:36] = (T0 - hi0.astype(np.float32)).astype(bf)
        out1[:, 0:4] = hi1
        out1[:, 32:36] = (T1 - hi1.astype(np.float32)).astype(bf)
        return out0, out1


def kernel(action_indecies, action_n_obj, action_types, W0, b0, W1, b1,
           **_unused):
    from concourse.bass_utils import run_bass_kernel_spmd

    idx = np.ascontiguousarray(np.asarray(action_indecies, dtype=np.int32))
    typ = np.ascontiguousarray(np.asarray(action_types, dtype=np.int32))
    B = idx.shape[0]
    b_core = B // N_CORES
    assert b_core * N_CORES == B

    tab0, tab1 = _tables(W0, b0, W1, b1, TABLE_MODE)
    selq = _selq(TABLE_MODE)

    key = (b_core, TABLE_MODE)
    if key not in _NC_CACHE:
        _NC_CACHE[key] = _build_nc(b_core, TABLE_MODE)
    nc, (SG, A_, K_, S) = _NC_CACHE[key]

    in_maps = [
        {"idx": idx[k * b_core:(k + 1) * b_core],
         "typ": typ[k * b_core:(k + 1) * b_core],
         "tab0": tab0, "tab1": tab1, "selq": selq}
        for k in range(N_CORES)
    ]
    res = run_bass_kernel_spmd(nc, in_maps, core_ids=list(range(N_CORES)))

    outs = []
    for r in res.results:
        o5 = r["out"]  # [SG, A, 4, K, S]
        o = np.transpose(o5, (0, 3, 1, 4, 2)).reshape(b_core, 4)
        outs.append(o)
    return np.ascontiguousarray(np.concatenate(outs, axis=0))
